# revision 1
# baseline (speedup 1.0000x reference)
"""Dilated attention Trainium2 kernel.

Problem: for each (batch, segment) pair, and each dilation rate r in {1,2,4,8}:
  q = Q_seg[::r], k = K_seg[::r], v = V_seg[::r]
  out_seg[::r] += softmax(q @ k.T) @ v        (no 1/sqrt(d) scaling)

Sharding: B=2 x n_seg=4 = 8 independent (batch, segment) pairs -> one per core.

Key trick (v3): the host uploads Q,K (pre-transposed to [d, l], fp16) and V
with their token axis PERMUTED as [j%8==0 | j%8==4 | j%4==2 | j odd].  Under
this order the rate-r token set {0, r, 2r, ...} is exactly the PREFIX
[0, 2048/r) — nested across rates.  Consequences:
  - every rate reads contiguous prefix slices of the SAME QT/KT/V tiles
    (no strided APs, one 4MB V buffer for all rates);
  - output rows for rate r are the prefix [0, 2048/r): all rates' PV
    matmuls for one output q-tile accumulate into ONE PSUM tile (no DRAM
    scratch, no combine pass, one eviction + store per output tile);
  - attention is permutation-invariant over the key set, so results are
    exact up to fp reordering; the host scatters rows back on download.

Per-core flow: output q-tiles processed DESCENDING (15..0), so the early
tiles touch only rate 1 (whose q-columns + KT arrive first in the split
DMA stream) and the multi-rate tiles (0,1) land at the end as a dense PE
tail.  Softmax uses block-local row maxes: each 512-col score block is
exp'ed immediately with its own max (PSUM bank frees right away -> scores
need 2 banks, PV output gets double buffering); the correction
exp(m_b - m)/rowsum is folded into one per-partition rescale of P before
the PE transposes, so PV emits O already normalized.
"""

import sys

if "/opt/trn_rl_repo" not in sys.path:
    sys.path.insert(0, "/opt/trn_rl_repo")

import numpy as np

import concourse.bass as bass
import concourse.mybir as mybir
from concourse import tile
from concourse.masks import make_identity
from concourse.bass_utils import run_bass_kernel_spmd

SEG_LEN = 2048
D = 1024
P = 128
NDCH = D // P  # 8 d-chunks of 128
BL = 512  # score block (PSUM bank) width
F16 = mybir.dt.float16
F32 = mybir.dt.float32

# token permutation: rate-r set {j : j % r == 0} -> prefix [0, 2048/r)
PERM = np.concatenate(
    [
        np.arange(0, SEG_LEN, 8),
        np.arange(4, SEG_LEN, 8),
        np.arange(2, SEG_LEN, 4),
        np.arange(1, SEG_LEN, 2),
    ]
)

# rates contributing to output q-tile t (descending r): prefix nesting
def tile_rates(t):
    rates = [1]
    if t < 8:
        rates.append(2)
    if t < 4:
        rates.append(4)
    if t < 2:
        rates.append(8)
    return rates


_ws_ctr = [0]


def _split_multi_waits(nc):
    """walrus in this env accepts only ONE sync-wait per instruction; move
    extras onto same-engine NoOps inserted right before the instruction."""
    for f in nc.m.functions:
        for b in f.blocks:
            out, changed = [], False
            for inst in b.instructions:
                si = inst.sync_info
                if si is not None and si.on_wait and len(si.on_wait) > 1:
                    waits = list(si.on_wait)
                    for w in waits[:-1]:
                        nop = mybir.InstNoOp(
                            name=f"waitsplit_{_ws_ctr[0]}", ins=[], outs=[]
                        )
                        _ws_ctr[0] += 1
                        nop.engine = inst.engine
                        nop.sync_info = mybir.SyncInfo(on_wait=[w], on_update=[])
                        out.append(nop)
                    si.on_wait = [waits[-1]]
                    changed = True
                out.append(inst)
            if changed:
                b.instructions = out


def build_kernel():
    # note: --enable-ldw-opt=true crashes the device (NRT_EXEC_UNIT_UNRECOVERABLE)
    # note: nc.scalar-issued xbar-transpose DMAs return wrong data in this env
    nc = bass.Bass()
    QTd = nc.dram_tensor("QT", (D, SEG_LEN), F16, kind="ExternalInput")
    KTd = nc.dram_tensor("KT", (D, SEG_LEN), F16, kind="ExternalInput")
    V = nc.dram_tensor("V", (SEG_LEN, D), F16, kind="ExternalInput")
    O = nc.dram_tensor("O", (SEG_LEN, D), F32, kind="ExternalOutput")

    with tile.TileContext(nc) as tc:
        with (
            tc.tile_pool(name="qkt", bufs=1) as qkt_pool,
            tc.tile_pool(name="pp1", bufs=5) as p1_pool,
            tc.tile_pool(name="pps", bufs=3) as ps_pool,
            tc.tile_pool(name="pt", bufs=18) as pt_pool,
            tc.tile_pool(name="op", bufs=3) as o_pool,
            tc.tile_pool(name="st", bufs=4) as stat_pool,
            tc.tile_pool(name="spsum", bufs=2, space="PSUM") as s_psum,
            tc.tile_pool(name="ptpsum", bufs=2, space="PSUM") as pt_psum,
            tc.tile_pool(name="opsum", bufs=2, space="PSUM") as o_psum,
        ):
            ident16 = qkt_pool.tile([P, P], F16, tag="ident", name="ident16")
            make_identity(nc, ident16[:])
            # dummy exp: forces the ACT engine's Exp table load to happen
            # during the startup preamble instead of before the first real
            # softmax exp
            warm = stat_pool.tile([P, 1], F32, tag="warm")
            nc.scalar.activation(
                warm[:], ident16[:, 0:1], mybir.ActivationFunctionType.Exp
            )

            QT = [
                qkt_pool.tile([P, SEG_LEN], F16, tag=f"QT{c}", name=f"QT{c}")
                for c in range(NDCH)
            ]
            KT = [
                qkt_pool.tile([P, SEG_LEN], F16, tag=f"KT{c}", name=f"KT{c}")
                for c in range(NDCH)
            ]
            Vt = qkt_pool.tile([P, 16, D], F16, tag="V", name="Vt")

            # ---- input DMA program, priority-ordered for DESCENDING q-tile
            # processing.  Round-robin over sync + gpsimd only: the ACT
            # engine must stay free for the exps (a front-loaded ACT DMA
            # program blocks on ring-full waits and starves the softmax,
            # which stalls the score-PSUM recycle and the PE).  ACT gets a
            # tiny early share (3 loads) and the late QT blocks, which are
            # sprinkled into the compute emission below.
            dma_engines = [nc.sync, nc.gpsimd]
            qctr = [0]

            def dq():
                e = dma_engines[qctr[0] % len(dma_engines)]
                qctr[0] += 1
                return e

            # 1) KT block 0 + QT block 3 (q-tile 15 starts on these)
            for c in range(NDCH):
                cs = slice(c * P, (c + 1) * P)
                (nc.scalar if c == 5 else dq()).dma_start(
                    KT[c][:, 0:BL], KTd[cs, 0:BL]
                )
                (nc.scalar if c in (4, 6) else dq()).dma_start(
                    QT[c][:, 3 * BL : 4 * BL], QTd[cs, 3 * BL : 4 * BL]
                )
            # ACT-dripped loads: issued one per score block from inside the
            # compute emission, so the ACT engine's issue cost interleaves
            # with the exps instead of queueing ahead of them.  Carries a
            # 1/4 share of the KT/V stream (an extra ~120GB/s during the
            # head) plus all of the late QT blocks.
            act_drip = []

            def drip_act():
                if act_drip:
                    dst, src = act_drip.pop(0)
                    nc.scalar.dma_start(dst, src)

            # 2) remaining KT blocks in score sweep order
            for b in range(1, SEG_LEN // BL):
                for c in range(NDCH):
                    cs = slice(c * P, (c + 1) * P)
                    pair = (
                        KT[c][:, b * BL : (b + 1) * BL],
                        KTd[cs, b * BL : (b + 1) * BL],
                    )
                    if c in (2, 5):
                        act_drip.append(pair)
                    else:
                        dq().dma_start(*pair)
            # 3) QT block 2 (q-tiles 11..8, needed ~35us) rides the ACT
            # drip -- the two main queues cannot deliver it in time
            # behind KT
            for c in range(NDCH):
                cs = slice(c * P, (c + 1) * P)
                act_drip.append(
                    (QT[c][:, 2 * BL : 3 * BL], QTd[cs, 2 * BL : 3 * BL])
                )
            for kt in range(16):
                pair = (Vt[:, kt, :], V[kt * P : (kt + 1) * P, :])
                if kt % 3 == 2:
                    act_drip.append(pair)
                else:
                    dq().dma_start(*pair)
            # late QT blocks (q-tiles 7..0), entirely ACT-dripped
            for b in (1, 0):
                for c in range(NDCH):
                    cs = slice(c * P, (c + 1) * P)
                    act_drip.append(
                        (QT[c][:, b * BL : (b + 1) * BL],
                         QTd[cs, b * BL : (b + 1) * BL])
                    )

            def emit_score_block(r, t, b, negpm, rsparts, Pt):
                L = SEG_LEN // r
                q0 = t * P
                n0 = b * BL
                n1 = min(L, n0 + BL)
                Sb = s_psum.tile([P, BL], F32, tag="S", name="Sb")
                for d in range(NDCH):
                    nc.tensor.matmul(
                        Sb[:, : n1 - n0],
                        QT[d][:, q0 : q0 + P],
                        KT[d][:, n0:n1],
                        start=(d == 0),
                        stop=(d == NDCH - 1),
                    )
                # block-local row max (negated -> exp bias), then exp right
                # away so the PSUM bank frees after ~1us
                nc.vector.tensor_reduce(
                    negpm[:, b : b + 1], Sb[:, : n1 - n0],
                    mybir.AxisListType.X, mybir.AluOpType.max, negate=True,
                )
                nc.scalar.activation(
                    Pt[:, n0:n1], Sb[:, : n1 - n0],
                    mybir.ActivationFunctionType.Exp,
                    bias=negpm[:, b : b + 1], scale=1.0,
                    accum_out=rsparts[:, b : b + 1],
                )
                drip_act()

            def emit_pair_scores(t_hi, t_lo):
                """Scores+exp for a PAIR of output tiles, block-major: both
                tiles' chains on one KT block run back to back, so during the
                head the PE has 2 chains (3.4us) of work per arriving KT
                block instead of 1.  Rate sets match within a pair."""
                stages = {
                    t: {"t": t, "subs": []} for t in (t_hi, t_lo)
                }
                for r in tile_rates(t_hi):
                    L = SEG_LEN // r
                    nblk = (L + BL - 1) // BL
                    subs = {}
                    for t in (t_hi, t_lo):
                        subs[t] = {
                            "r": r,
                            "negpm": stat_pool.tile(
                                [P, 4], F32, tag=f"negpm{r}_{t % 2}",
                                name=f"negpm{r}_{t % 2}",
                            ),
                            "rsparts": stat_pool.tile(
                                [P, 4], F32, tag=f"rsparts{r}_{t % 2}",
                                name=f"rsparts{r}_{t % 2}",
                            ),
                            "Pt": (p1_pool if r == 1 else ps_pool).tile(
                                [P, L], F16, tag=f"P{r}_{t % 2}",
                                name=f"P{r}_{t}",
                            ),
                            "nblk": nblk,
                        }
                        stages[t]["subs"].append(subs[t])
                    for b in range(nblk):
                        for t in (t_hi, t_lo):
                            s = subs[t]
                            emit_score_block(
                                r, t, b, s["negpm"], s["rsparts"], s["Pt"]
                            )
                # finalize immediately: the DVE/ACT rescales overlap the
                # next PE work instead of gating the PV transposes later
                for t in (t_hi, t_lo):
                    for sub in stages[t]["subs"]:
                        emit_finalize_softmax(sub)
                return [stages[t_hi], stages[t_lo]]

            def emit_finalize_softmax(sub):
                """Fold the block-max correction exp(m_b - m) and the softmax
                normalization 1/rowsum into one per-partition rescale of P.
                After this, PV produces this rate's contribution normalized."""
                if sub.get("finalized"):
                    return
                sub["finalized"] = True
                nblk, L = sub["nblk"], SEG_LEN // sub["r"]
                negpm, rsparts, Pt = sub["negpm"], sub["rsparts"], sub["Pt"]
                rinv = stat_pool.tile([P, 1], F32, tag="rinv")
                if nblk == 1:
                    nc.vector.reciprocal(rinv[:], rsparts[:, 0:1])
                    h = L // 2
                    nc.vector.tensor_scalar_mul(Pt[:, :h], Pt[:, :h], rinv[:])
                    nc.scalar.mul(Pt[:, h:], Pt[:, h:], rinv[:])
                    return
                # negm = -max_b(m_b) = min_b(negm_b)
                negm = stat_pool.tile([P, 1], F32, tag="negm")
                nc.vector.tensor_reduce(
                    negm[:], negpm[:, :nblk],
                    mybir.AxisListType.X, mybir.AluOpType.min,
                )
                # c_b = exp(m_b - m) = exp(-negm_b + negm)
                cb = stat_pool.tile([P, 4], F32, tag="cb")
                nc.scalar.activation(
                    cb[:, :nblk], negpm[:, :nblk],
                    mybir.ActivationFunctionType.Exp,
                    bias=negm[:], scale=-1.0,
                )
                # rowsum = sum_b c_b * rs_b  (one fused DVE op)
                cr = stat_pool.tile([P, 4], F32, tag="cr")
                rowsum = stat_pool.tile([P, 1], F32, tag="rowsum")
                nc.vector.scalar_tensor_tensor(
                    cr[:, :nblk], cb[:, :nblk], 1.0, rsparts[:, :nblk],
                    mybir.AluOpType.mult, mybir.AluOpType.mult,
                    accum_out=rowsum[:],
                )
                nc.vector.reciprocal(rinv[:], rowsum[:])
                cs = stat_pool.tile([P, 4], F32, tag="cs")
                nc.vector.tensor_scalar_mul(cs[:, :nblk], cb[:, :nblk], rinv[:])
                for b in range(nblk):
                    n0, n1 = b * BL, min(L, (b + 1) * BL)
                    blk = Pt[:, n0:n1]
                    if b % 2 == 0:
                        nc.vector.tensor_scalar_mul(blk, blk, cs[:, b : b + 1])
                    else:
                        nc.scalar.mul(blk, blk, cs[:, b : b + 1])

            deferred_evict = []

            def emit_evict(ev):
                Ops, t = ev
                Osb = o_pool.tile([P, D], F32, tag="Osb")
                nc.vector.tensor_copy(Osb[:, :512], Ops[:, :512])
                nc.scalar.copy(Osb[:, 512:], Ops[:, 512:])
                nc.sync.dma_start(O[t * P : (t + 1) * P, 0:512], Osb[:, 0:512])
                nc.gpsimd.dma_start(
                    O[t * P : (t + 1) * P, 512:1024], Osb[:, 512:1024]
                )

            def emit_pv(stg):
                """One PSUM accumulation chain over ALL rates of tile t, then
                a single eviction + store."""
                t = stg["t"]
                Ops = o_psum.tile([P, D], F32, tag="O")
                n_chunks = sum(SEG_LEN // s["r"] // P for s in stg["subs"])
                chunk = [0]

                def emit_one_pv(pts, kt, knt):
                    first = chunk[0] == 0
                    chunk[0] += 1
                    last = chunk[0] == n_chunks
                    for n0 in (0, 512):
                        nc.tensor.matmul(
                            Ops[:, n0 : n0 + 512],
                            pts[kt][:],
                            Vt[:, knt, n0 : n0 + 512],
                            start=first,
                            stop=last,
                        )

                # per rate: interleave transposes and (2-behind) PV matmuls
                # so a transpose waiting on its eviction never head-of-line
                # blocks ready PV work on the PE
                for sub in stg["subs"]:
                    emit_finalize_softmax(sub)
                    Pt = sub["Pt"]
                    n_kt = SEG_LEN // sub["r"] // P
                    pts = []
                    for kt in range(n_kt):
                        ptp = pt_psum.tile([P, P], F16, tag="ptp", name="pp2")
                        nc.tensor.transpose(
                            ptp[:], Pt[:, kt * P : (kt + 1) * P], ident16[:]
                        )
                        ptsb = pt_pool.tile([P, P], F16, tag="pts")
                        if kt % 2 == 0:
                            nc.vector.tensor_copy(ptsb[:], ptp[:])
                        else:
                            nc.scalar.copy(ptsb[:], ptp[:])
                        pts.append(ptsb)
                        if kt >= 2:
                            emit_one_pv(pts, kt - 2, kt - 2)
                    for kt in range(max(0, n_kt - 2), n_kt):
                        emit_one_pv(pts, kt, kt)

                # O is already normalized.  The eviction is DEFERRED: it is
                # emitted during the NEXT PV stage (o_psum is double
                # buffered), so its PSUM->SBUF copies never queue ahead of
                # the next PV's P^T-copy chain on DVE/ACT.
                deferred_evict.append((Ops, t))
                if len(deferred_evict) > 1:
                    emit_evict(deferred_evict.pop(0))

            # software pipeline: PV runs DEPTH output-tiles behind scores so
            # the early PVs don't race the V DMA stream and the softmax tail
            # hides under later scores
            # tapered pipeline: the first four pairs of scores are emitted
            # before ANY PV, banking ~54us of PE work so the V stream (which
            # shares the 2.5 DMA queues with K/Q) fully lands before the
            # first PV needs it; afterwards the pipeline narrows to 4 tiles
            pops_before_pair = [0, 0, 0, 0, 4, 2, 2, 2]
            pending = []
            for k, t_hi in enumerate(range(15, 0, -2)):
                for _ in range(pops_before_pair[k]):
                    emit_pv(pending.pop(0))
                pending.extend(emit_pair_scores(t_hi, t_hi - 1))
            for stg in pending:
                emit_pv(stg)
            # trailing eviction: 3-way store split to minimize the exposed
            # post-matmul store latency at the very end of the kernel
            for Ops, t in deferred_evict:
                Osb = o_pool.tile([P, D], F32, tag="Osb")
                nc.vector.tensor_copy(Osb[:, :512], Ops[:, :512])
                nc.scalar.copy(Osb[:, 512:], Ops[:, 512:])
                rows = slice(t * P, (t + 1) * P)
                nc.sync.dma_start(O[rows, 0:512], Osb[:, 0:512])
                nc.gpsimd.dma_start(O[rows, 512:768], Osb[:, 512:768])
                nc.scalar.dma_start(O[rows, 768:1024], Osb[:, 768:1024])

    _split_multi_waits(nc)
    return nc


_NC_CACHE = None


def make_in_maps(Q, K, V):
    """Shard + permute + cast the full inputs into per-core upload dicts."""
    n_seg = Q.shape[1] // SEG_LEN
    in_maps = []
    for c in range(8):
        b, g = divmod(c, n_seg)
        sl = slice(g * SEG_LEN, (g + 1) * SEG_LEN)
        in_maps.append(
            {
                "QT": np.ascontiguousarray(
                    Q[b, sl].T[:, PERM], dtype=np.float16
                ),
                "KT": np.ascontiguousarray(
                    K[b, sl].T[:, PERM], dtype=np.float16
                ),
                "V": np.ascontiguousarray(V[b, sl][PERM, :], dtype=np.float16),
            }
        )
    return in_maps


def unshard(results, B, S, Dm):
    n_seg = S // SEG_LEN
    out = np.empty((B, S, Dm), dtype=np.float32)
    for c in range(8):
        b, g = divmod(c, n_seg)
        seg = np.empty((SEG_LEN, Dm), dtype=np.float32)
        seg[PERM, :] = results[c]["O"]
        out[b, g * SEG_LEN : (g + 1) * SEG_LEN, :] = seg
    return out


def kernel(Q, K, V):
    global _NC_CACHE
    Q = np.asarray(Q)
    K = np.asarray(K)
    V = np.asarray(V)
    B, S, Dm = Q.shape
    assert (B, S, Dm) == (2, 8192, 1024)

    if _NC_CACHE is None:
        _NC_CACHE = build_kernel()
    nc = _NC_CACHE

    res = run_bass_kernel_spmd(
        nc, make_in_maps(Q, K, V), core_ids=list(range(8))
    )
    return unshard(res.results, B, S, Dm)


if __name__ == "__main__":
    rng = np.random.default_rng(0)
    Q = rng.standard_normal((2, 8192, 1024), dtype=np.float32)
    K = rng.standard_normal((2, 8192, 1024), dtype=np.float32)
    V = rng.standard_normal((2, 8192, 1024), dtype=np.float32)
    out = kernel(Q=Q, K=K, V=V)
    print("ran ok", out.shape, out.dtype, np.abs(out).mean())



# revision 2
# speedup vs baseline: 1.3024x; 1.3024x over previous
"""Dilated attention Trainium2 kernel (v4: cross-rate score dedup).

Problem: for each (batch, segment) pair, and each dilation rate r in {1,2,4,8}:
  q = Q_seg[::r], k = K_seg[::r], v = V_seg[::r]
  out_seg[::r] += softmax(q @ k.T) @ v        (no 1/sqrt(d) scaling)

Sharding: B=2 x n_seg=4 = 8 independent (batch, segment) pairs -> one per core.

Host-side permutation (as in v3): tokens reordered as
[j%8==0 | j%8==4 | j%4==2 | j odd], so the rate-r token set is the PREFIX
[0, 2048/r).

Key upgrade over v3: since S[i,j] = q_i . k_j is rate independent, rate r's
score matrix is exactly the leading (2048/r)^2 block of rate 1's.  So per
q-tile we compute the 2048-wide score row-block ONCE, exp it ONCE (with
block-local row maxes over 5 column groups: 256,256,512,512,512), and fold
ALL applicable rates' softmax normalizations into a single per-(row,group)
weight:
    out[i] = sum_j E[i,j] * W[i, g(j)] * v_j,
    W[i,g] = sum_{rates r: L_r > max(i-group, j-group)} exp(m_g - m_r) / Z_r
One transpose+PV chain per q-tile then yields the summed-over-rates output
directly.  PE work drops ~25% vs v3 (no duplicated score/exp/PV blocks).

Schedule: scores for tiles 15..12 emitted block-major as KT blocks stream
in, then pairs (11,10),(9,8) banked (first PV waits until V fully lands),
then steady-state [pair scores + 2 PVs], then an 8-PV tail.  Evictions are
deferred one PV like v3.
"""

import sys

if "/opt/trn_rl_repo" not in sys.path:
    sys.path.insert(0, "/opt/trn_rl_repo")

import numpy as np

import concourse.bass as bass
import concourse.mybir as mybir
from concourse import tile
from concourse.masks import make_identity
from concourse.bass_utils import run_bass_kernel_spmd

SEG_LEN = 2048
D = 1024
P = 128
NDCH = D // P  # 8 d-chunks of 128
BL = 512  # score block (PSUM bank) width
F16 = mybir.dt.float16
F32 = mybir.dt.float32

# column groups for block-local softmax stats (rate boundaries 256/512/1024)
GROUPS = ((0, 256), (256, 512), (512, 1024), (1024, 1536), (1536, 2048))
NGROUP = {1: 5, 2: 3, 4: 2, 8: 1}  # prefix group count per rate

# token permutation: rate-r set {j : j % r == 0} -> prefix [0, 2048/r)
PERM = np.concatenate(
    [
        np.arange(0, SEG_LEN, 8),
        np.arange(4, SEG_LEN, 8),
        np.arange(2, SEG_LEN, 4),
        np.arange(1, SEG_LEN, 2),
    ]
)


# rates contributing to output q-tile t: prefix nesting
def tile_rates(t):
    rates = [1]
    if t < 8:
        rates.append(2)
    if t < 4:
        rates.append(4)
    if t < 2:
        rates.append(8)
    return rates


_ws_ctr = [0]


def _split_multi_waits(nc):
    """walrus in this env accepts only ONE sync-wait per instruction; move
    extras onto same-engine NoOps inserted right before the instruction."""
    for f in nc.m.functions:
        for b in f.blocks:
            out, changed = [], False
            for inst in b.instructions:
                si = inst.sync_info
                if si is not None and si.on_wait and len(si.on_wait) > 1:
                    waits = list(si.on_wait)
                    for w in waits[:-1]:
                        nop = mybir.InstNoOp(
                            name=f"waitsplit_{_ws_ctr[0]}", ins=[], outs=[]
                        )
                        _ws_ctr[0] += 1
                        nop.engine = inst.engine
                        nop.sync_info = mybir.SyncInfo(on_wait=[w], on_update=[])
                        out.append(nop)
                    si.on_wait = [waits[-1]]
                    changed = True
                out.append(inst)
            if changed:
                b.instructions = out


def build_kernel():
    # note: --enable-ldw-opt=true crashes the device (NRT_EXEC_UNIT_UNRECOVERABLE)
    # note: nc.scalar-issued xbar-transpose DMAs return wrong data in this env
    nc = bass.Bass()
    QTd = nc.dram_tensor("QT", (D, SEG_LEN), F16, kind="ExternalInput")
    KTd = nc.dram_tensor("KT", (D, SEG_LEN), F16, kind="ExternalInput")
    V = nc.dram_tensor("V", (SEG_LEN, D), F16, kind="ExternalInput")
    O = nc.dram_tensor("O", (SEG_LEN, D), F32, kind="ExternalOutput")

    Exp = mybir.ActivationFunctionType.Exp
    AX = mybir.AxisListType.X
    MAX = mybir.AluOpType.max
    MIN = mybir.AluOpType.min
    MULT = mybir.AluOpType.mult
    ADD = mybir.AluOpType.add

    with tile.TileContext(nc) as tc:
        with (
            tc.tile_pool(name="qkt", bufs=1) as qkt_pool,
            tc.tile_pool(name="pp", bufs=10) as p_pool,
            tc.tile_pool(name="pt", bufs=18) as pt_pool,
            tc.tile_pool(name="op", bufs=3) as o_pool,
            tc.tile_pool(name="st", bufs=2) as stat_pool,
            tc.tile_pool(name="spsum", bufs=2, space="PSUM") as s_psum,
            tc.tile_pool(name="ptpsum", bufs=2, space="PSUM") as pt_psum,
            tc.tile_pool(name="opsum", bufs=2, space="PSUM") as o_psum,
        ):
            QT = [
                qkt_pool.tile([P, SEG_LEN], F16, tag=f"QT{c}", name=f"QT{c}")
                for c in range(NDCH)
            ]
            KT = [
                qkt_pool.tile([P, SEG_LEN], F16, tag=f"KT{c}", name=f"KT{c}")
                for c in range(NDCH)
            ]
            Vt = qkt_pool.tile([P, 16, D], F16, tag="V", name="Vt")

            # ---- input DMA program.  All issued up front on sync/gpsimd
            # (plus a small scalar share for the critical head loads); deep
            # score banking below makes every later deadline lax, so no
            # exp-interleaved drip is needed.
            dma3 = [nc.sync, nc.gpsimd, nc.scalar]
            dma2 = [nc.sync, nc.gpsimd]
            ctr3, ctr2 = [0], [0]

            def dq3():
                e = dma3[ctr3[0] % 3]
                ctr3[0] += 1
                return e

            def dq2():
                e = dma2[ctr2[0] % 2]
                ctr2[0] += 1
                return e

            # phase 1a: KT block 0 (first score chains touch it)
            for c in range(NDCH):
                cs = slice(c * P, (c + 1) * P)
                dq3().dma_start(KT[c][:, 0:BL], KTd[cs, 0:BL])
            # phase 1b: QT columns for tile 15 (narrow, lands fast)
            for c in range(NDCH):
                cs = slice(c * P, (c + 1) * P)
                dq3().dma_start(
                    QT[c][:, 15 * P : 16 * P], QTd[cs, 15 * P : 16 * P]
                )
            # phase 1c: QT columns for tiles 14..12
            for c in range(NDCH):
                cs = slice(c * P, (c + 1) * P)
                dq2().dma_start(
                    QT[c][:, 12 * P : 15 * P], QTd[cs, 12 * P : 15 * P]
                )

            # identity (for PE transposes, needed ~60us in) + exp table warm
            ident16 = qkt_pool.tile([P, P], F16, tag="ident", name="ident16")
            make_identity(nc, ident16[:])
            warm_in = stat_pool.tile([P, 1], F32, tag="warm_in", name="warm_in")
            nc.vector.memset(warm_in[:], 0.0)
            warm = stat_pool.tile([P, 1], F32, tag="warm", name="warm")
            nc.scalar.activation(warm[:], warm_in[:], Exp)

            # phase 2: KT blocks 1..3, then QT block 2, then V, then QT 1,0
            for b in range(1, 4):
                for c in range(NDCH):
                    cs = slice(c * P, (c + 1) * P)
                    dq2().dma_start(
                        KT[c][:, b * BL : (b + 1) * BL],
                        KTd[cs, b * BL : (b + 1) * BL],
                    )
            for c in range(NDCH):
                cs = slice(c * P, (c + 1) * P)
                dq2().dma_start(
                    QT[c][:, 2 * BL : 3 * BL], QTd[cs, 2 * BL : 3 * BL]
                )
            for kt in range(16):
                dq2().dma_start(Vt[:, kt, :], V[kt * P : (kt + 1) * P, :])
            for b in (1, 0):
                for c in range(NDCH):
                    cs = slice(c * P, (c + 1) * P)
                    dq2().dma_start(
                        QT[c][:, b * BL : (b + 1) * BL],
                        QTd[cs, b * BL : (b + 1) * BL],
                    )

            # ---- score block: matmul chain + block-local max + exp
            def emit_score_block(t, b, st):
                q0 = t * P
                n0 = b * BL
                Sb = s_psum.tile([P, BL], F32, tag="S", name="Sb")
                for d in range(NDCH):
                    nc.tensor.matmul(
                        Sb[:],
                        QT[d][:, q0 : q0 + P],
                        KT[d][:, n0 : n0 + BL],
                        start=(d == 0),
                        stop=(d == NDCH - 1),
                    )
                ng, rs, Pt = st["ng"], st["rs"], st["Pt"]
                if b == 0:
                    # two 256-wide half groups (rate-8/4 boundaries)
                    for g in (0, 1):
                        sl = slice(g * 256, (g + 1) * 256)
                        nc.vector.tensor_reduce(
                            ng[:, g : g + 1], Sb[:, sl], AX, MAX, negate=True
                        )
                        nc.scalar.activation(
                            Pt[:, sl], Sb[:, sl], Exp,
                            bias=ng[:, g : g + 1], scale=1.0,
                            accum_out=rs[:, g : g + 1],
                        )
                else:
                    g = b + 1
                    nc.vector.tensor_reduce(
                        ng[:, g : g + 1], Sb[:], AX, MAX, negate=True
                    )
                    nc.scalar.activation(
                        Pt[:, n0 : n0 + BL], Sb[:], Exp,
                        bias=ng[:, g : g + 1], scale=1.0,
                        accum_out=rs[:, g : g + 1],
                    )

            def new_stage(t):
                return {
                    "t": t,
                    "ng": stat_pool.tile([P, 8], F32, tag="ng", bufs=6, name="ng"),
                    "rs": stat_pool.tile([P, 8], F32, tag="rs", bufs=6, name="rs"),
                    "Pt": p_pool.tile([P, SEG_LEN], F16, tag="P", name="Pt"),
                }

            # ---- finalize: per-rate Z from group stats, combined weights,
            # one rescale of Pt per group -> PV output is sum over rates,
            # already normalized
            def emit_finalize(st):
                t, ng, rs, Pt = st["t"], st["ng"], st["rs"], st["Pt"]
                rates = tile_rates(t)
                nr = len(rates)
                Z = stat_pool.tile([P, 4], F32, tag="Z", name="Z")
                cbs = {}
                for ri, r in enumerate(rates):
                    gn = NGROUP[r]
                    if gn == 1:
                        nc.vector.tensor_copy(Z[:, ri : ri + 1], rs[:, 0:1])
                        continue
                    negm = stat_pool.tile([P, 1], F32, tag="negm", name="negm")
                    nc.vector.tensor_reduce(negm[:], ng[:, :gn], AX, MIN)
                    cb = stat_pool.tile([P, 8], F32, tag=f"cb{ri}", name=f"cb{ri}")
                    nc.scalar.activation(
                        cb[:, :gn], ng[:, :gn], Exp, bias=negm[:], scale=-1.0
                    )
                    cr = stat_pool.tile([P, 8], F32, tag=f"cr{ri}", name=f"cr{ri}")
                    nc.vector.scalar_tensor_tensor(
                        cr[:, :gn], cb[:, :gn], 1.0, rs[:, :gn],
                        MULT, MULT, accum_out=Z[:, ri : ri + 1],
                    )
                    cbs[ri] = cb
                rinv = stat_pool.tile([P, 4], F32, tag="rinv", name="rinv")
                nc.vector.reciprocal(rinv[:, :nr], Z[:, :nr])
                W = stat_pool.tile([P, 8], F32, tag="W", bufs=4, name="W")
                nc.vector.tensor_scalar_mul(W[:, :5], cbs[0][:, :5], rinv[:, 0:1])
                for ri, r in list(enumerate(rates))[1:]:
                    gn = NGROUP[r]
                    if gn == 1:
                        nc.vector.tensor_scalar_add(
                            W[:, 0:1], W[:, 0:1], rinv[:, ri : ri + 1]
                        )
                    else:
                        nc.vector.scalar_tensor_tensor(
                            W[:, :gn], cbs[ri][:, :gn], rinv[:, ri : ri + 1],
                            W[:, :gn], MULT, ADD,
                        )
                for gi, (g0, g1) in enumerate(GROUPS):
                    blk = Pt[:, g0:g1]
                    if gi % 2 == 0:
                        nc.vector.tensor_scalar_mul(blk, blk, W[:, gi : gi + 1])
                    else:
                        nc.scalar.mul(blk, blk, W[:, gi : gi + 1])

            deferred_evict = []

            def emit_evict(ev):
                Ops, t = ev
                Osb = o_pool.tile([P, D], F32, tag="Osb", name="Osb")
                nc.vector.tensor_copy(Osb[:, :512], Ops[:, :512])
                nc.scalar.copy(Osb[:, 512:], Ops[:, 512:])
                nc.sync.dma_start(O[t * P : (t + 1) * P, 0:512], Osb[:, 0:512])
                nc.gpsimd.dma_start(
                    O[t * P : (t + 1) * P, 512:1024], Osb[:, 512:1024]
                )

            # ---- PV: 16 transposes + one PSUM accumulation chain over all
            # 2048 (weighted) columns, transposes running 2 ahead of PV
            def emit_pv(st):
                t, Pt = st["t"], st["Pt"]
                Ops = o_psum.tile([P, D], F32, tag="O", name="Ops")

                def emit_one_pv(pts, kt):
                    first = kt == 0
                    last = kt == 15
                    for n0 in (0, 512):
                        nc.tensor.matmul(
                            Ops[:, n0 : n0 + 512],
                            pts[kt][:],
                            Vt[:, kt, n0 : n0 + 512],
                            start=first,
                            stop=last,
                        )

                pts = []
                for kt in range(16):
                    ptp = pt_psum.tile([P, P], F16, tag="ptp", name="ptp")
                    nc.tensor.transpose(
                        ptp[:], Pt[:, kt * P : (kt + 1) * P], ident16[:]
                    )
                    ptsb = pt_pool.tile([P, P], F16, tag="pts", name="pts")
                    if kt % 2 == 0:
                        nc.vector.tensor_copy(ptsb[:], ptp[:])
                    else:
                        nc.scalar.copy(ptsb[:], ptp[:])
                    pts.append(ptsb)
                    if kt >= 2:
                        emit_one_pv(pts, kt - 2)
                emit_one_pv(pts, 14)
                emit_one_pv(pts, 15)

                # deferred eviction: emitted during the NEXT PV (o_psum is
                # double buffered) so its copies never queue ahead of the
                # next PV's P^T-copy chain on DVE/ACT
                deferred_evict.append((Ops, t))
                if len(deferred_evict) > 1:
                    emit_evict(deferred_evict.pop(0))

            # ---- schedule
            pending = []

            # head quad 15..12, block-major: PE progresses as KT blocks land
            quad = [new_stage(t) for t in (15, 14, 13, 12)]
            for b in range(4):
                for st in quad:
                    emit_score_block(st["t"], b, st)
                    if b == 3:
                        emit_finalize(st)
            pending.extend(quad)

            # bank two more pairs (first PV must wait for the full V stream)
            for t_hi in (11, 9):
                sts = [new_stage(t_hi), new_stage(t_hi - 1)]
                for b in range(4):
                    for st in sts:
                        emit_score_block(st["t"], b, st)
                for st in sts:
                    emit_finalize(st)
                pending.extend(sts)

            # steady state: pair scores + 2 PVs
            for t_hi in (7, 5, 3, 1):
                sts = [new_stage(t_hi), new_stage(t_hi - 1)]
                for b in range(4):
                    for st in sts:
                        emit_score_block(st["t"], b, st)
                for st in sts:
                    emit_finalize(st)
                emit_pv(pending.pop(0))
                emit_pv(pending.pop(0))
                pending.extend(sts)

            # tail: remaining PVs
            for st in pending:
                emit_pv(st)
            # trailing eviction(s): 3-way store split to minimize exposed
            # post-matmul store latency at the very end
            for Ops, t in deferred_evict:
                Osb = o_pool.tile([P, D], F32, tag="Osb", name="Osb")
                nc.vector.tensor_copy(Osb[:, :512], Ops[:, :512])
                nc.scalar.copy(Osb[:, 512:], Ops[:, 512:])
                rows = slice(t * P, (t + 1) * P)
                nc.sync.dma_start(O[rows, 0:512], Osb[:, 0:512])
                nc.gpsimd.dma_start(O[rows, 512:768], Osb[:, 512:768])
                nc.scalar.dma_start(O[rows, 768:1024], Osb[:, 768:1024])

    _split_multi_waits(nc)
    return nc


_NC_CACHE = None


def make_in_maps(Q, K, V):
    """Shard + permute + cast the full inputs into per-core upload dicts."""
    n_seg = Q.shape[1] // SEG_LEN
    in_maps = []
    for c in range(8):
        b, g = divmod(c, n_seg)
        sl = slice(g * SEG_LEN, (g + 1) * SEG_LEN)
        in_maps.append(
            {
                "QT": np.ascontiguousarray(
                    Q[b, sl].T[:, PERM], dtype=np.float16
                ),
                "KT": np.ascontiguousarray(
                    K[b, sl].T[:, PERM], dtype=np.float16
                ),
                "V": np.ascontiguousarray(V[b, sl][PERM, :], dtype=np.float16),
            }
        )
    return in_maps


def unshard(results, B, S, Dm):
    n_seg = S // SEG_LEN
    out = np.empty((B, S, Dm), dtype=np.float32)
    for c in range(8):
        b, g = divmod(c, n_seg)
        seg = np.empty((SEG_LEN, Dm), dtype=np.float32)
        seg[PERM, :] = results[c]["O"]
        out[b, g * SEG_LEN : (g + 1) * SEG_LEN, :] = seg
    return out


def kernel(Q, K, V):
    global _NC_CACHE
    Q = np.asarray(Q)
    K = np.asarray(K)
    V = np.asarray(V)
    B, S, Dm = Q.shape
    assert (B, S, Dm) == (2, 8192, 1024)

    if _NC_CACHE is None:
        _NC_CACHE = build_kernel()
    nc = _NC_CACHE

    res = run_bass_kernel_spmd(
        nc, make_in_maps(Q, K, V), core_ids=list(range(8))
    )
    return unshard(res.results, B, S, Dm)


if __name__ == "__main__":
    rng = np.random.default_rng(0)
    Q = rng.standard_normal((2, 8192, 1024), dtype=np.float32)
    K = rng.standard_normal((2, 8192, 1024), dtype=np.float32)
    V = rng.standard_normal((2, 8192, 1024), dtype=np.float32)
    out = kernel(Q=Q, K=K, V=V)
    print("ran ok", out.shape, out.dtype, np.abs(out).mean())


# revision 6
# speedup vs baseline: 1.3358x; 1.0257x over previous
"""Dilated attention Trainium2 kernel (v5: dedup + packed DMA + gpsimd evict).

Problem: for each (batch, segment) pair, and each dilation rate r in {1,2,4,8}:
  q = Q_seg[::r], k = K_seg[::r], v = V_seg[::r]
  out_seg[::r] += softmax(q @ k.T) @ v        (no 1/sqrt(d) scaling)

Sharding: B=2 x n_seg=4 = 8 independent (batch, segment) pairs -> one per core.

Host-side permutation (as in v3): tokens reordered as
[j%8==0 | j%8==4 | j%4==2 | j odd], so the rate-r token set is the PREFIX
[0, 2048/r).

v4 upgrade: since S[i,j] = q_i . k_j is rate independent, rate r's score
matrix is exactly the leading (2048/r)^2 block of rate 1's.  Per q-tile the
2048-wide score row-block is computed ONCE, exp'ed ONCE (block-local row
maxes over 5 column groups: 256,256,512,512,512), and ALL applicable rates'
softmax normalizations fold into one per-(row,group) weight:
    out[i] = sum_j E[i,j] * W[i, g(j)] * v_j,
    W[i,g] = sum_{rates r covering row i and group g} exp(m_g - m_r) / Z_r
One transpose+PV chain per q-tile yields the summed-over-rates output.
PE work drops ~25% vs v3.

v5 upgrades (scheduling only):
  - Host packs Q/K/V so each SBUF destination loads with ONE wide dma_start
    (2-8KB per partition row): ~22 input DMAs instead of ~112.  Kills the
    DMA-issue serialization at the head (~570ns per dma_start on the issue
    engine) and lands KT block 0 several us earlier.
  - PSUM->SBUF output eviction of PV n-1 is emitted mid-PV-n (after the
    kt==5 transpose-copy), so the o_psum bank recycles one full PV before
    the PV that reuses it and its copies slot into DVE/ACT idle time.  The
    old len>1 deferral freed the bank only inside the PV that needed it
    (~1us PE gap per PV in the tail).  (GPSIMD cannot access PSUM on this
    target, so evictions stay on DVE/ACT.)
  - Each pair's PVs are emitted BEFORE the pair's softmax-finalize ops so
    the PV copy chain never queues behind finalize work on DVE/ACT.
"""

import sys

if "/opt/trn_rl_repo" not in sys.path:
    sys.path.insert(0, "/opt/trn_rl_repo")

import numpy as np

import concourse.bass as bass
import concourse.mybir as mybir
from concourse import tile
from concourse.masks import make_identity
from concourse.bass_utils import run_bass_kernel_spmd

SEG_LEN = 2048
D = 1024
P = 128
NDCH = D // P  # 8 d-chunks of 128
BL = 512  # score block (PSUM bank) width
NBLK = SEG_LEN // BL  # 4
NKT = SEG_LEN // P  # 16 k-tiles
F16 = mybir.dt.float16
F32 = mybir.dt.float32

# column groups for block-local softmax stats (rate boundaries 256/512/1024)
GROUPS = ((0, 256), (256, 512), (512, 1024), (1024, 1536), (1536, 2048))
NGROUP = {1: 5, 2: 3, 4: 2, 8: 1}  # prefix group count per rate

# token permutation: rate-r set {j : j % r == 0} -> prefix [0, 2048/r)
PERM = np.concatenate(
    [
        np.arange(0, SEG_LEN, 8),
        np.arange(4, SEG_LEN, 8),
        np.arange(2, SEG_LEN, 4),
        np.arange(1, SEG_LEN, 2),
    ]
)


# rates contributing to output q-tile t: prefix nesting
def tile_rates(t):
    rates = [1]
    if t < 8:
        rates.append(2)
    if t < 4:
        rates.append(4)
    if t < 2:
        rates.append(8)
    return rates


_ws_ctr = [0]


def _split_multi_waits(nc):
    """walrus in this env accepts only ONE sync-wait per instruction; move
    extras onto same-engine NoOps inserted right before the instruction."""
    for f in nc.m.functions:
        for b in f.blocks:
            out, changed = [], False
            for inst in b.instructions:
                si = inst.sync_info
                if si is not None and si.on_wait and len(si.on_wait) > 1:
                    waits = list(si.on_wait)
                    for w in waits[:-1]:
                        nop = mybir.InstNoOp(
                            name=f"waitsplit_{_ws_ctr[0]}", ins=[], outs=[]
                        )
                        _ws_ctr[0] += 1
                        nop.engine = inst.engine
                        nop.sync_info = mybir.SyncInfo(on_wait=[w], on_update=[])
                        out.append(nop)
                    si.on_wait = [waits[-1]]
                    changed = True
                out.append(inst)
            if changed:
                b.instructions = out


def build_kernel():
    # note: --enable-ldw-opt=true crashes the device (NRT_EXEC_UNIT_UNRECOVERABLE)
    # note: nc.scalar-issued xbar-transpose DMAs return wrong data in this env
    nc = bass.Bass()
    # packed layouts (see make_in_maps):
    #   Qp[r, t, c, j] = Q^T[c*128+r, t*128+j]   (tile-major q columns)
    #   Kp[r, b, c, j] = K^T[c*128+r, b*512+j]   (block-major k columns)
    #   Vp[r, kt, j]   = V[kt*128+r, j]
    Qp = nc.dram_tensor("QP", (P, NKT, NDCH, P), F16, kind="ExternalInput")
    Kp = nc.dram_tensor("KP", (P, NBLK, NDCH, BL), F16, kind="ExternalInput")
    Vp = nc.dram_tensor("VP", (P, NKT, D), F16, kind="ExternalInput")
    O = nc.dram_tensor("O", (SEG_LEN, D), F32, kind="ExternalOutput")

    Exp = mybir.ActivationFunctionType.Exp
    AX = mybir.AxisListType.X
    MAX = mybir.AluOpType.max
    MIN = mybir.AluOpType.min
    MULT = mybir.AluOpType.mult
    ADD = mybir.AluOpType.add

    with tile.TileContext(nc) as tc:
        with (
            tc.tile_pool(name="qkt", bufs=1) as qkt_pool,
            tc.tile_pool(name="pp", bufs=10) as p_pool,
            tc.tile_pool(name="pt", bufs=18) as pt_pool,
            tc.tile_pool(name="op", bufs=3) as o_pool,
            tc.tile_pool(name="st", bufs=2) as stat_pool,
            tc.tile_pool(name="spsum", bufs=2, space="PSUM") as s_psum,
            tc.tile_pool(name="ptpsum", bufs=2, space="PSUM") as pt_psum,
            tc.tile_pool(name="opsum", bufs=2, space="PSUM") as o_psum,
        ):
            QTs = qkt_pool.tile([P, NKT, NDCH, P], F16, tag="QT", name="QTs")
            KTs = qkt_pool.tile([P, NBLK, NDCH, BL], F16, tag="KT", name="KTs")
            Vt = qkt_pool.tile([P, NKT, D], F16, tag="V", name="Vt")

            # ---- input DMA program: few, wide dma_starts.  sync carries the
            # KT stream, gpsimd the QT stream + V; scalar stays free for exps.
            # priority: KT block 0 + first 4 q-tiles (head), then KT blocks,
            # then next q-tiles, then V, then remaining q-tiles.
            nc.sync.dma_start(KTs[:, 0], Kp[:, 0])
            for t in (15, 14, 13, 12):
                nc.gpsimd.dma_start(QTs[:, t], Qp[:, t])
            for b in (1, 2, 3):
                nc.sync.dma_start(KTs[:, b], Kp[:, b])
            for t in (11, 10, 9, 8):
                nc.gpsimd.dma_start(QTs[:, t], Qp[:, t])
            # V in 4 parts, alternating queues
            for i in range(4):
                e = nc.sync if i % 2 == 0 else nc.gpsimd
                e.dma_start(Vt[:, 4 * i : 4 * i + 4, :], Vp[:, 4 * i : 4 * i + 4, :])
            for i, t in enumerate((7, 6, 5, 4, 3, 2, 1, 0)):
                e = nc.sync if i % 2 == 0 else nc.gpsimd
                e.dma_start(QTs[:, t], Qp[:, t])

            # identity (for PE transposes, needed ~55us in) + exp table warm
            ident16 = qkt_pool.tile([P, P], F16, tag="ident", name="ident16")
            make_identity(nc, ident16[:])
            warm_in = stat_pool.tile([P, 1], F32, tag="warm_in", name="warm_in")
            nc.vector.memset(warm_in[:], 0.0)
            warm = stat_pool.tile([P, 1], F32, tag="warm", name="warm")
            nc.scalar.activation(warm[:], warm_in[:], Exp)

            # ---- score block: matmul chain + block-local max + exp
            def emit_score_block(t, b, st):
                Sb = s_psum.tile([P, BL], F32, tag="S", name="Sb")
                for d in range(NDCH):
                    nc.tensor.matmul(
                        Sb[:],
                        QTs[:, t, d, :],
                        KTs[:, b, d, :],
                        start=(d == 0),
                        stop=(d == NDCH - 1),
                    )
                ng, rs, Pt = st["ng"], st["rs"], st["Pt"]
                if b == 0:
                    # two 256-wide half groups (rate-8/4 boundaries)
                    for g in (0, 1):
                        sl = slice(g * 256, (g + 1) * 256)
                        nc.vector.tensor_reduce(
                            ng[:, g : g + 1], Sb[:, sl], AX, MAX, negate=True
                        )
                        nc.scalar.activation(
                            Pt[:, sl], Sb[:, sl], Exp,
                            bias=ng[:, g : g + 1], scale=1.0,
                            accum_out=rs[:, g : g + 1],
                        )
                else:
                    g = b + 1
                    nc.vector.tensor_reduce(
                        ng[:, g : g + 1], Sb[:], AX, MAX, negate=True
                    )
                    nc.scalar.activation(
                        Pt[:, b * BL : (b + 1) * BL], Sb[:], Exp,
                        bias=ng[:, g : g + 1], scale=1.0,
                        accum_out=rs[:, g : g + 1],
                    )

            def new_stage(t):
                return {
                    "t": t,
                    "ng": stat_pool.tile([P, 8], F32, tag="ng", bufs=6, name="ng"),
                    "rs": stat_pool.tile([P, 8], F32, tag="rs", bufs=6, name="rs"),
                    "Pt": p_pool.tile([P, SEG_LEN], F16, tag="P", name="Pt"),
                }

            # ---- finalize: per-rate Z from group stats, combined weights,
            # one rescale of Pt per group -> PV output is sum over rates,
            # already normalized
            def emit_finalize(st):
                t, ng, rs, Pt = st["t"], st["ng"], st["rs"], st["Pt"]
                rates = tile_rates(t)
                nr = len(rates)
                Z = stat_pool.tile([P, 4], F32, tag="Z", name="Z")
                cbs = {}
                for ri, r in enumerate(rates):
                    gn = NGROUP[r]
                    if gn == 1:
                        nc.vector.tensor_copy(Z[:, ri : ri + 1], rs[:, 0:1])
                        continue
                    negm = stat_pool.tile([P, 1], F32, tag="negm", name="negm")
                    nc.vector.tensor_reduce(negm[:], ng[:, :gn], AX, MIN)
                    cb = stat_pool.tile([P, 8], F32, tag=f"cb{ri}", name=f"cb{ri}")
                    nc.scalar.activation(
                        cb[:, :gn], ng[:, :gn], Exp, bias=negm[:], scale=-1.0
                    )
                    cr = stat_pool.tile([P, 8], F32, tag=f"cr{ri}", name=f"cr{ri}")
                    nc.vector.scalar_tensor_tensor(
                        cr[:, :gn], cb[:, :gn], 1.0, rs[:, :gn],
                        MULT, MULT, accum_out=Z[:, ri : ri + 1],
                    )
                    cbs[ri] = cb
                rinv = stat_pool.tile([P, 4], F32, tag="rinv", name="rinv")
                nc.vector.reciprocal(rinv[:, :nr], Z[:, :nr])
                W = stat_pool.tile([P, 8], F32, tag="W", bufs=4, name="W")
                nc.vector.tensor_scalar_mul(W[:, :5], cbs[0][:, :5], rinv[:, 0:1])
                for ri, r in list(enumerate(rates))[1:]:
                    gn = NGROUP[r]
                    if gn == 1:
                        nc.vector.tensor_scalar_add(
                            W[:, 0:1], W[:, 0:1], rinv[:, ri : ri + 1]
                        )
                    else:
                        nc.vector.scalar_tensor_tensor(
                            W[:, :gn], cbs[ri][:, :gn], rinv[:, ri : ri + 1],
                            W[:, :gn], MULT, ADD,
                        )
                for gi, (g0, g1) in enumerate(GROUPS):
                    blk = Pt[:, g0:g1]
                    if gi % 2 == 0:
                        nc.vector.tensor_scalar_mul(blk, blk, W[:, gi : gi + 1])
                    else:
                        nc.scalar.mul(blk, blk, W[:, gi : gi + 1])

            deferred_evict = []

            def emit_evict(ev, final=False):
                Ops, t = ev
                Osb = o_pool.tile([P, D], F32, tag="Osb", name="Osb")
                rows = slice(t * P, (t + 1) * P)
                if final:
                    nc.vector.tensor_copy(Osb[:, :512], Ops[:, :512])
                    nc.scalar.copy(Osb[:, 512:], Ops[:, 512:])
                    nc.sync.dma_start(O[rows, 0:512], Osb[:, 0:512])
                    nc.gpsimd.dma_start(O[rows, 512:768], Osb[:, 512:768])
                    nc.scalar.dma_start(O[rows, 768:1024], Osb[:, 768:1024])
                else:
                    nc.vector.tensor_copy(Osb[:, :512], Ops[:, :512])
                    nc.scalar.copy(Osb[:, 512:], Ops[:, 512:])
                    nc.sync.dma_start(O[rows, 0:512], Osb[:, 0:512])
                    nc.gpsimd.dma_start(O[rows, 512:1024], Osb[:, 512:1024])

            # ---- PV: 16 transposes + one PSUM accumulation chain over all
            # 2048 (weighted) columns, transposes running 2 ahead of PV
            def emit_pv(st):
                t, Pt = st["t"], st["Pt"]
                Ops = o_psum.tile([P, D], F32, tag="O", name="Ops")

                def emit_one_pv(pts, kt):
                    first = kt == 0
                    last = kt == NKT - 1
                    for n0 in (0, 512):
                        nc.tensor.matmul(
                            Ops[:, n0 : n0 + 512],
                            pts[kt][:],
                            Vt[:, kt, n0 : n0 + 512],
                            start=first,
                            stop=last,
                        )

                pts = []
                for kt in range(NKT):
                    ptp = pt_psum.tile([P, P], F16, tag="ptp", name="ptp")
                    nc.tensor.transpose(
                        ptp[:], Pt[:, kt * P : (kt + 1) * P], ident16[:]
                    )
                    ptsb = pt_pool.tile([P, P], F16, tag="pts", name="pts")
                    if kt % 2 == 0:
                        nc.vector.tensor_copy(ptsb[:], ptp[:])
                    else:
                        nc.scalar.copy(ptsb[:], ptp[:])
                    pts.append(ptsb)
                    if kt == 5 and deferred_evict:
                        # recycle the PREVIOUS PV's o_psum bank mid-PV: its
                        # copies slot into the DVE/ACT idle time behind the
                        # P^T copy chain, so the NEXT PV's first matmul
                        # (which reuses that bank) never waits
                        emit_evict(deferred_evict.pop(0))
                    if kt >= 2:
                        emit_one_pv(pts, kt - 2)
                emit_one_pv(pts, NKT - 2)
                emit_one_pv(pts, NKT - 1)
                deferred_evict.append((Ops, t))

            # ---- schedule
            pending = []

            # head quad 15..12, block-major: PE progresses as KT blocks land
            quad = [new_stage(t) for t in (15, 14, 13, 12)]
            for b in range(NBLK):
                for st in quad:
                    emit_score_block(st["t"], b, st)
                    if b == NBLK - 1:
                        emit_finalize(st)
            pending.extend(quad)

            # bank one more pair (first PV waits for the full V stream)
            sts = [new_stage(11), new_stage(10)]
            for b in range(NBLK):
                for st in sts:
                    emit_score_block(st["t"], b, st)
            for st in sts:
                emit_finalize(st)
            pending.extend(sts)

            # steady state: pair scores, then 2 PVs, then the pair's
            # finalizes (so PV copies never queue behind finalize work)
            for t_hi in (9, 7, 5, 3, 1):
                sts = [new_stage(t_hi), new_stage(t_hi - 1)]
                for b in range(NBLK):
                    for st in sts:
                        emit_score_block(st["t"], b, st)
                emit_pv(pending.pop(0))
                emit_pv(pending.pop(0))
                for st in sts:
                    emit_finalize(st)
                pending.extend(sts)

            # tail: remaining PVs
            for st in pending:
                emit_pv(st)
            # trailing evictions: last one 3-way split to minimize exposed
            # post-matmul store latency
            while deferred_evict:
                emit_evict(deferred_evict.pop(0), final=(len(deferred_evict) == 0))

    _split_multi_waits(nc)
    return nc


_NC_CACHE = None


def make_in_maps(Q, K, V):
    """Shard + permute + cast + pack the full inputs into per-core uploads."""
    n_seg = Q.shape[1] // SEG_LEN
    in_maps = []
    for c in range(8):
        b, g = divmod(c, n_seg)
        sl = slice(g * SEG_LEN, (g + 1) * SEG_LEN)
        QT = Q[b, sl].T[:, PERM].astype(np.float16)  # [1024, 2048]
        KT = K[b, sl].T[:, PERM].astype(np.float16)
        Vs = V[b, sl][PERM, :].astype(np.float16)  # [2048, 1024]
        in_maps.append(
            {
                # [128, 16, 8, 128]: partition r, q-tile t, d-chunk c, col j
                "QP": np.ascontiguousarray(
                    QT.reshape(NDCH, P, NKT, P).transpose(1, 2, 0, 3)
                ),
                # [128, 4, 8, 512]: partition r, k-block b, d-chunk c, col j
                "KP": np.ascontiguousarray(
                    KT.reshape(NDCH, P, NBLK, BL).transpose(1, 2, 0, 3)
                ),
                # [128, 16, 1024]: partition r, k-tile kt, d col
                "VP": np.ascontiguousarray(
                    Vs.reshape(NKT, P, D).transpose(1, 0, 2)
                ),
            }
        )
    return in_maps


def unshard(results, B, S, Dm):
    n_seg = S // SEG_LEN
    out = np.empty((B, S, Dm), dtype=np.float32)
    for c in range(8):
        b, g = divmod(c, n_seg)
        seg = np.empty((SEG_LEN, Dm), dtype=np.float32)
        seg[PERM, :] = results[c]["O"]
        out[b, g * SEG_LEN : (g + 1) * SEG_LEN, :] = seg
    return out


def kernel(Q, K, V):
    global _NC_CACHE
    Q = np.asarray(Q)
    K = np.asarray(K)
    V = np.asarray(V)
    B, S, Dm = Q.shape
    assert (B, S, Dm) == (2, 8192, 1024)

    if _NC_CACHE is None:
        _NC_CACHE = build_kernel()
    nc = _NC_CACHE

    res = run_bass_kernel_spmd(
        nc, make_in_maps(Q, K, V), core_ids=list(range(8))
    )
    return unshard(res.results, B, S, Dm)


if __name__ == "__main__":
    rng = np.random.default_rng(0)
    Q = rng.standard_normal((2, 8192, 1024), dtype=np.float32)
    K = rng.standard_normal((2, 8192, 1024), dtype=np.float32)
    V = rng.standard_normal((2, 8192, 1024), dtype=np.float32)
    out = kernel(Q=Q, K=K, V=V)
    print("ran ok", out.shape, out.dtype, np.abs(out).mean())


# revision 10
# speedup vs baseline: 1.3530x; 1.0129x over previous
"""Dilated attention Trainium2 kernel (v5: dedup + packed DMA + gpsimd evict).

Problem: for each (batch, segment) pair, and each dilation rate r in {1,2,4,8}:
  q = Q_seg[::r], k = K_seg[::r], v = V_seg[::r]
  out_seg[::r] += softmax(q @ k.T) @ v        (no 1/sqrt(d) scaling)

Sharding: B=2 x n_seg=4 = 8 independent (batch, segment) pairs -> one per core.

Host-side permutation (as in v3): tokens reordered as
[j%8==0 | j%8==4 | j%4==2 | j odd], so the rate-r token set is the PREFIX
[0, 2048/r).

v4 upgrade: since S[i,j] = q_i . k_j is rate independent, rate r's score
matrix is exactly the leading (2048/r)^2 block of rate 1's.  Per q-tile the
2048-wide score row-block is computed ONCE, exp'ed ONCE (block-local row
maxes over 5 column groups: 256,256,512,512,512), and ALL applicable rates'
softmax normalizations fold into one per-(row,group) weight:
    out[i] = sum_j E[i,j] * W[i, g(j)] * v_j,
    W[i,g] = sum_{rates r covering row i and group g} exp(m_g - m_r) / Z_r
One transpose+PV chain per q-tile yields the summed-over-rates output.
PE work drops ~25% vs v3.

v5 upgrades (scheduling only):
  - Host packs Q/K/V so each SBUF destination loads with ONE wide dma_start
    (2-8KB per partition row): ~22 input DMAs instead of ~112.  Kills the
    DMA-issue serialization at the head (~570ns per dma_start on the issue
    engine) and lands KT block 0 several us earlier.
  - PSUM->SBUF output eviction of PV n-1 is emitted mid-PV-n (after the
    kt==5 transpose-copy), so the o_psum bank recycles one full PV before
    the PV that reuses it and its copies slot into DVE/ACT idle time.  The
    old len>1 deferral freed the bank only inside the PV that needed it
    (~1us PE gap per PV in the tail).  (GPSIMD cannot access PSUM on this
    target, so evictions stay on DVE/ACT.)
  - Each pair's PVs are emitted BEFORE the pair's softmax-finalize ops so
    the PV copy chain never queues behind finalize work on DVE/ACT.
"""

import sys

if "/opt/trn_rl_repo" not in sys.path:
    sys.path.insert(0, "/opt/trn_rl_repo")

import numpy as np

import concourse.bass as bass
import concourse.mybir as mybir
from concourse import tile
from concourse.masks import make_identity
from concourse.bass_utils import run_bass_kernel_spmd

SEG_LEN = 2048
D = 1024
P = 128
NDCH = D // P  # 8 d-chunks of 128
BL = 512  # score block (PSUM bank) width
NBLK = SEG_LEN // BL  # 4
NKT = SEG_LEN // P  # 16 k-tiles
F16 = mybir.dt.float16
F32 = mybir.dt.float32

# column groups for block-local softmax stats (rate boundaries 256/512/1024)
GROUPS = ((0, 256), (256, 512), (512, 1024), (1024, 1536), (1536, 2048))
NGROUP = {1: 5, 2: 3, 4: 2, 8: 1}  # prefix group count per rate

# token permutation: rate-r set {j : j % r == 0} -> prefix [0, 2048/r)
PERM = np.concatenate(
    [
        np.arange(0, SEG_LEN, 8),
        np.arange(4, SEG_LEN, 8),
        np.arange(2, SEG_LEN, 4),
        np.arange(1, SEG_LEN, 2),
    ]
)


# rates contributing to output q-tile t: prefix nesting
def tile_rates(t):
    rates = [1]
    if t < 8:
        rates.append(2)
    if t < 4:
        rates.append(4)
    if t < 2:
        rates.append(8)
    return rates


_ws_ctr = [0]


def _split_multi_waits(nc):
    """walrus in this env accepts only ONE sync-wait per instruction; move
    extras onto same-engine NoOps inserted right before the instruction."""
    for f in nc.m.functions:
        for b in f.blocks:
            out, changed = [], False
            for inst in b.instructions:
                si = inst.sync_info
                if si is not None and si.on_wait and len(si.on_wait) > 1:
                    waits = list(si.on_wait)
                    for w in waits[:-1]:
                        nop = mybir.InstNoOp(
                            name=f"waitsplit_{_ws_ctr[0]}", ins=[], outs=[]
                        )
                        _ws_ctr[0] += 1
                        nop.engine = inst.engine
                        nop.sync_info = mybir.SyncInfo(on_wait=[w], on_update=[])
                        out.append(nop)
                    si.on_wait = [waits[-1]]
                    changed = True
                out.append(inst)
            if changed:
                b.instructions = out


def build_kernel():
    # note: --enable-ldw-opt=true crashes the device (NRT_EXEC_UNIT_UNRECOVERABLE)
    # note: nc.scalar-issued xbar-transpose DMAs return wrong data in this env
    nc = bass.Bass()
    # packed layouts (see make_in_maps):
    #   Qp[r, t, c, j] = Q^T[c*128+r, t*128+j]   (tile-major q columns)
    #   Kp[r, b, c, j] = K^T[c*128+r, b*512+j]   (block-major k columns)
    #   Vp[r, kt, j]   = V[kt*128+r, j]
    Qp = nc.dram_tensor("QP", (P, NKT, NDCH, P), F16, kind="ExternalInput")
    Kp = nc.dram_tensor("KP", (P, NBLK, NDCH, BL), F16, kind="ExternalInput")
    Vp = nc.dram_tensor("VP", (P, NKT, D), F16, kind="ExternalInput")
    O = nc.dram_tensor("O", (SEG_LEN, D), F32, kind="ExternalOutput")

    Exp = mybir.ActivationFunctionType.Exp
    AX = mybir.AxisListType.X
    MAX = mybir.AluOpType.max
    MIN = mybir.AluOpType.min
    MULT = mybir.AluOpType.mult
    ADD = mybir.AluOpType.add

    with tile.TileContext(nc) as tc:
        with (
            tc.tile_pool(name="qkt", bufs=1) as qkt_pool,
            tc.tile_pool(name="pp", bufs=10) as p_pool,
            tc.tile_pool(name="pt", bufs=18) as pt_pool,
            tc.tile_pool(name="op", bufs=6) as o_pool,
            tc.tile_pool(name="st", bufs=2) as stat_pool,
            tc.tile_pool(name="spsum", bufs=2, space="PSUM") as s_psum,
            tc.tile_pool(name="ptpsum", bufs=2, space="PSUM") as pt_psum,
            tc.tile_pool(name="opsum", bufs=2, space="PSUM") as o_psum,
        ):
            QTs = qkt_pool.tile([P, NKT, NDCH, P], F16, tag="QT", name="QTs")
            KTs = qkt_pool.tile([P, NBLK, NDCH, BL], F16, tag="KT", name="KTs")
            Vt = qkt_pool.tile([P, NKT, D], F16, tag="V", name="Vt")

            # ---- input DMA program: few, wide dma_starts.  sync carries the
            # KT stream, gpsimd the QT stream + V; scalar stays free for exps.
            # priority: KT block 0 + first 4 q-tiles (head), then KT blocks,
            # then next q-tiles, then V, then remaining q-tiles.
            # KT block 0 in quarters (2 d-chunks each) so the first score
            # chain's early matmuls start as soon as their chunks land;
            # subtile deps let matmul d wait only on its own quarter
            nc.sync.dma_start(KTs[:, 0, 0:2], Kp[:, 0, 0:2])
            nc.scalar.dma_start(KTs[:, 0, 2:4], Kp[:, 0, 2:4])
            nc.sync.dma_start(KTs[:, 0, 4:6], Kp[:, 0, 4:6])
            nc.scalar.dma_start(KTs[:, 0, 6:8], Kp[:, 0, 6:8])
            for t in (15, 14, 13, 12):
                nc.gpsimd.dma_start(QTs[:, t], Qp[:, t])
            for b in (1, 2, 3):
                nc.sync.dma_start(KTs[:, b], Kp[:, b])
            for t in (11, 10, 9, 8):
                nc.gpsimd.dma_start(QTs[:, t], Qp[:, t])
            # V in 4 parts, alternating queues
            for i in range(4):
                e = nc.sync if i % 2 == 0 else nc.gpsimd
                e.dma_start(Vt[:, 4 * i : 4 * i + 4, :], Vp[:, 4 * i : 4 * i + 4, :])
            for i, t in enumerate((7, 6, 5, 4, 3, 2, 1, 0)):
                e = nc.sync if i % 2 == 0 else nc.gpsimd
                e.dma_start(QTs[:, t], Qp[:, t])

            # identity (for PE transposes, needed ~55us in) + exp table warm
            ident16 = qkt_pool.tile([P, P], F16, tag="ident", name="ident16")
            make_identity(nc, ident16[:])
            warm_in = stat_pool.tile([P, 1], F32, tag="warm_in", name="warm_in")
            nc.vector.memset(warm_in[:], 0.0)
            warm = stat_pool.tile([P, 1], F32, tag="warm", name="warm")
            nc.scalar.activation(warm[:], warm_in[:], Exp)

            # ---- score block: matmul chain + block-local max + exp
            def emit_score_block(t, b, st):
                Sb = s_psum.tile([P, BL], F32, tag="S", name="Sb")
                for d in range(NDCH):
                    nc.tensor.matmul(
                        Sb[:],
                        QTs[:, t, d, :],
                        KTs[:, b, d, :],
                        start=(d == 0),
                        stop=(d == NDCH - 1),
                    )
                ng, rs, Pt = st["ng"], st["rs"], st["Pt"]
                if b == 0:
                    # two 256-wide half groups (rate-8/4 boundaries)
                    for g in (0, 1):
                        sl = slice(g * 256, (g + 1) * 256)
                        nc.vector.tensor_reduce(
                            ng[:, g : g + 1], Sb[:, sl], AX, MAX, negate=True
                        )
                        nc.scalar.activation(
                            Pt[:, sl], Sb[:, sl], Exp,
                            bias=ng[:, g : g + 1], scale=1.0,
                            accum_out=rs[:, g : g + 1],
                        )
                else:
                    g = b + 1
                    nc.vector.tensor_reduce(
                        ng[:, g : g + 1], Sb[:], AX, MAX, negate=True
                    )
                    nc.scalar.activation(
                        Pt[:, b * BL : (b + 1) * BL], Sb[:], Exp,
                        bias=ng[:, g : g + 1], scale=1.0,
                        accum_out=rs[:, g : g + 1],
                    )

            def new_stage(t):
                return {
                    "t": t,
                    "ng": stat_pool.tile([P, 8], F32, tag="ng", bufs=6, name="ng"),
                    "rs": stat_pool.tile([P, 8], F32, tag="rs", bufs=6, name="rs"),
                    "Pt": p_pool.tile([P, SEG_LEN], F16, tag="P", name="Pt"),
                }

            # ---- finalize: per-rate Z from group stats, combined weights,
            # one rescale of Pt per group -> PV output is sum over rates,
            # already normalized
            def emit_finalize(st):
                t, ng, rs, Pt = st["t"], st["ng"], st["rs"], st["Pt"]
                rates = tile_rates(t)
                nr = len(rates)
                Z = stat_pool.tile([P, 4], F32, tag="Z", name="Z")
                cbs = {}
                for ri, r in enumerate(rates):
                    gn = NGROUP[r]
                    if gn == 1:
                        nc.vector.tensor_copy(Z[:, ri : ri + 1], rs[:, 0:1])
                        continue
                    negm = stat_pool.tile([P, 1], F32, tag="negm", name="negm")
                    nc.vector.tensor_reduce(negm[:], ng[:, :gn], AX, MIN)
                    cb = stat_pool.tile([P, 8], F32, tag=f"cb{ri}", name=f"cb{ri}")
                    nc.scalar.activation(
                        cb[:, :gn], ng[:, :gn], Exp, bias=negm[:], scale=-1.0
                    )
                    cr = stat_pool.tile([P, 8], F32, tag=f"cr{ri}", name=f"cr{ri}")
                    nc.vector.scalar_tensor_tensor(
                        cr[:, :gn], cb[:, :gn], 1.0, rs[:, :gn],
                        MULT, MULT, accum_out=Z[:, ri : ri + 1],
                    )
                    cbs[ri] = cb
                rinv = stat_pool.tile([P, 4], F32, tag="rinv", name="rinv")
                nc.vector.reciprocal(rinv[:, :nr], Z[:, :nr])
                W = stat_pool.tile([P, 8], F32, tag="W", bufs=4, name="W")
                nc.vector.tensor_scalar_mul(W[:, :5], cbs[0][:, :5], rinv[:, 0:1])
                for ri, r in list(enumerate(rates))[1:]:
                    gn = NGROUP[r]
                    if gn == 1:
                        nc.vector.tensor_scalar_add(
                            W[:, 0:1], W[:, 0:1], rinv[:, ri : ri + 1]
                        )
                    else:
                        nc.vector.scalar_tensor_tensor(
                            W[:, :gn], cbs[ri][:, :gn], rinv[:, ri : ri + 1],
                            W[:, :gn], MULT, ADD,
                        )
                for gi, (g0, g1) in enumerate(GROUPS):
                    blk = Pt[:, g0:g1]
                    if gi % 2 == 0:
                        nc.vector.tensor_scalar_mul(blk, blk, W[:, gi : gi + 1])
                    else:
                        nc.scalar.mul(blk, blk, W[:, gi : gi + 1])

            deferred_evict = []

            def emit_evict(ev, final=False):
                Ops, t = ev
                Osb = o_pool.tile([P, D], F32, tag="Osb", name="Osb")
                rows = slice(t * P, (t + 1) * P)
                if final:
                    nc.vector.tensor_copy(Osb[:, :512], Ops[:, :512])
                    nc.scalar.copy(Osb[:, 512:], Ops[:, 512:])
                    nc.sync.dma_start(O[rows, 0:512], Osb[:, 0:512])
                    nc.gpsimd.dma_start(O[rows, 512:768], Osb[:, 512:768])
                    nc.scalar.dma_start(O[rows, 768:1024], Osb[:, 768:1024])
                else:
                    nc.vector.tensor_copy(Osb[:, :512], Ops[:, :512])
                    nc.scalar.copy(Osb[:, 512:], Ops[:, 512:])
                    nc.sync.dma_start(O[rows, 0:512], Osb[:, 0:512])
                    nc.gpsimd.dma_start(O[rows, 512:1024], Osb[:, 512:1024])

            # ---- PV: 16 transposes + one PSUM accumulation chain over all
            # 2048 (weighted) columns, transposes running 2 ahead of PV
            def transpose_copy(Pt, kt):
                ptp = pt_psum.tile([P, P], F16, tag="ptp", name="ptp")
                nc.tensor.transpose(
                    ptp[:], Pt[:, kt * P : (kt + 1) * P], ident16[:]
                )
                ptsb = pt_pool.tile([P, P], F16, tag="pts", name="pts")
                if kt % 2 == 0:
                    nc.vector.tensor_copy(ptsb[:], ptp[:])
                else:
                    nc.scalar.copy(ptsb[:], ptp[:])
                return ptsb

            def pre_pv(st):
                """Emit the first two transpose+copies of st's PV ahead of
                time (end of the previous PV / mid-scores), so at the PV
                boundary the PE's kt2 transpose and kt0 matmul never wait on
                the DVE/ACT copy queues."""
                if st is None or "pre" in st:
                    return
                st["pre"] = [transpose_copy(st["Pt"], kt) for kt in (0, 1)]

            def emit_pv(st, nxt=None):
                t, Pt = st["t"], st["Pt"]
                Ops = o_psum.tile([P, D], F32, tag="O", name="Ops")

                def emit_one_pv(pts, kt):
                    first = kt == 0
                    last = kt == NKT - 1
                    for n0 in (0, 512):
                        nc.tensor.matmul(
                            Ops[:, n0 : n0 + 512],
                            pts[kt][:],
                            Vt[:, kt, n0 : n0 + 512],
                            start=first,
                            stop=last,
                        )

                pts = list(st.get("pre", ()))
                for kt in range(len(pts), NKT):
                    pts.append(transpose_copy(Pt, kt))
                    if kt == 5 and deferred_evict:
                        # recycle the PREVIOUS PV's o_psum bank mid-PV: its
                        # copies slot into the DVE/ACT idle time behind the
                        # P^T copy chain, so the NEXT PV's first matmul
                        # (which reuses that bank) never waits
                        emit_evict(deferred_evict.pop(0))
                    if kt >= 2:
                        emit_one_pv(pts, kt - 2)
                emit_one_pv(pts, NKT - 2)
                emit_one_pv(pts, NKT - 1)
                pre_pv(nxt)
                deferred_evict.append((Ops, t))

            # ---- schedule
            pending = []

            # head quad 15..12, block-major: PE progresses as KT blocks land
            quad = [new_stage(t) for t in (15, 14, 13, 12)]
            for b in range(NBLK):
                for st in quad:
                    emit_score_block(st["t"], b, st)
                    if b == NBLK - 1:
                        emit_finalize(st)
            pending.extend(quad)

            # bank one more pair (first PV waits for the full V stream)
            sts = [new_stage(11), new_stage(10)]
            for b in range(NBLK):
                for st in sts:
                    emit_score_block(st["t"], b, st)
            for st in sts:
                emit_finalize(st)
            pending.extend(sts)

            # steady state: pair scores, then 2 PVs, then the pair's
            # finalizes (so PV copies never queue behind finalize work)
            for t_hi in (9, 7, 5, 3, 1):
                sts = [new_stage(t_hi), new_stage(t_hi - 1)]
                for b in range(NBLK):
                    if b == NBLK - 1:
                        pre_pv(pending[0])
                    for st in sts:
                        emit_score_block(st["t"], b, st)
                a = pending.pop(0)
                b2 = pending.pop(0)
                emit_pv(a, nxt=b2)
                emit_pv(b2, nxt=pending[0] if pending else None)
                for st in sts:
                    emit_finalize(st)
                pending.extend(sts)

            # tail: remaining PVs
            for i, st in enumerate(pending):
                emit_pv(st, nxt=pending[i + 1] if i + 1 < len(pending) else None)
            # trailing evictions: last one 3-way split to minimize exposed
            # post-matmul store latency
            while deferred_evict:
                emit_evict(deferred_evict.pop(0), final=(len(deferred_evict) == 0))

    _split_multi_waits(nc)
    return nc


_NC_CACHE = None


def make_in_maps(Q, K, V):
    """Shard + permute + cast + pack the full inputs into per-core uploads."""
    n_seg = Q.shape[1] // SEG_LEN
    in_maps = []
    for c in range(8):
        b, g = divmod(c, n_seg)
        sl = slice(g * SEG_LEN, (g + 1) * SEG_LEN)
        QT = Q[b, sl].T[:, PERM].astype(np.float16)  # [1024, 2048]
        KT = K[b, sl].T[:, PERM].astype(np.float16)
        Vs = V[b, sl][PERM, :].astype(np.float16)  # [2048, 1024]
        in_maps.append(
            {
                # [128, 16, 8, 128]: partition r, q-tile t, d-chunk c, col j
                "QP": np.ascontiguousarray(
                    QT.reshape(NDCH, P, NKT, P).transpose(1, 2, 0, 3)
                ),
                # [128, 4, 8, 512]: partition r, k-block b, d-chunk c, col j
                "KP": np.ascontiguousarray(
                    KT.reshape(NDCH, P, NBLK, BL).transpose(1, 2, 0, 3)
                ),
                # [128, 16, 1024]: partition r, k-tile kt, d col
                "VP": np.ascontiguousarray(
                    Vs.reshape(NKT, P, D).transpose(1, 0, 2)
                ),
            }
        )
    return in_maps


def unshard(results, B, S, Dm):
    n_seg = S // SEG_LEN
    out = np.empty((B, S, Dm), dtype=np.float32)
    for c in range(8):
        b, g = divmod(c, n_seg)
        seg = np.empty((SEG_LEN, Dm), dtype=np.float32)
        seg[PERM, :] = results[c]["O"]
        out[b, g * SEG_LEN : (g + 1) * SEG_LEN, :] = seg
    return out


def kernel(Q, K, V):
    global _NC_CACHE
    Q = np.asarray(Q)
    K = np.asarray(K)
    V = np.asarray(V)
    B, S, Dm = Q.shape
    assert (B, S, Dm) == (2, 8192, 1024)

    if _NC_CACHE is None:
        _NC_CACHE = build_kernel()
    nc = _NC_CACHE

    res = run_bass_kernel_spmd(
        nc, make_in_maps(Q, K, V), core_ids=list(range(8))
    )
    return unshard(res.results, B, S, Dm)


if __name__ == "__main__":
    rng = np.random.default_rng(0)
    Q = rng.standard_normal((2, 8192, 1024), dtype=np.float32)
    K = rng.standard_normal((2, 8192, 1024), dtype=np.float32)
    V = rng.standard_normal((2, 8192, 1024), dtype=np.float32)
    out = kernel(Q=Q, K=K, V=V)
    print("ran ok", out.shape, out.dtype, np.abs(out).mean())
